# revision 13
# baseline (speedup 1.0000x reference)
"""Trainium2 Bass kernel for nn_AttnAutoEncoderRNN (H=1024, V=50257, T=256).

Strategy (v2):
  - GRU recurrence replicated on all 8 cores (collective floor ~10-20us/step
    rules out per-step cross-core sync); the [V,H] projection + log_softmax
    vocab-sharded 8 ways with one tiny AllReduce of the exp-sums.
  - Recurrence weights fp8-e4m3 with an exact power-of-2 scale S=256 folded
    into host-scaled biases and activation `scale=` immediates. fp8 matters:
    the N=1 LDW+MM pair rate is ~21ns for fp8 vs ~38ns bf16 (FWL is
    LDW-bound, 4 vs 2 elems/cycle) — measured on this device.
  - The serial gate/softmax chains (~300-400ns per cross-engine hop) are
    hidden under the matmul streams:
      * per-step PSUM bias/x-part preloads via fp16 identity matmuls replace
        the memset and all standalone bias-add DVE ops;
      * streams are ordered kc-outer so the first 64 pairs of a step only
        need the first half of h; the gate tail produces h in two halves,
        letting the next step's stream start while the second half finishes;
      * attention: exp accumulates under the Whh stream; the exp-sum
        (SM-scaled ones-matmul) reciprocal scales e BEFORE the e@M2 matmul
        (e_norm), so q lands on a pc-preloaded PSUM and a single relu (split
        in halves) is all that gates the Wih stream.
  - sigmoid(x) = 0.5*tanh(x/2)+0.5 keeps the recurrence+exp in one ACT
    table set; r and z share one fused [128,16] tanh.
  - M2 = enc_outs @ C2.T precomputed once (SM-scaled fp8), so ctx folds into
    e @ M2. Projection runs ONE matmul pass; logits cached fp16 in SBUF;
    after the exp-sum AllReduce only a subtract+DMA pass remains.
"""

import numpy as np
import ml_dtypes

import concourse.bass as bass
import concourse.bacc as bacc
import concourse.tile as tile
import concourse.mybir as mybir
from concourse.bass_utils import run_bass_kernel_spmd

BF16 = ml_dtypes.bfloat16
FP8 = ml_dtypes.float8_e4m3
F16NP = np.float16
F32 = mybir.dt.float32
F16 = mybir.dt.float16
BF = mybir.dt.bfloat16
E4 = mybir.dt.float8e4
AF = mybir.ActivationFunctionType
ALU = mybir.AluOpType

H = 1024
HC = H // 128            # 8 k-chunks of the hidden dim
HH = HC // 2             # half of the k-chunks
G = 3 * H                # 3072 gate rows
GC = G // 128            # 24 gate m-tiles
V_FULL = 50257
N_CORES = 8
SOS = 1
NV = 512                 # vocab tile width in the projection
S = 256.0                # fp8 weight scale (power of 2: exact on bf16/fp32)
SM = 32.0                # fp8 scale for the M2 attention matrix


def _cdiv(a, b):
    return (a + b - 1) // b


def build_program(T, VS, VA, reps=1, n_cores=N_CORES, debug_outs=False,
                  no_collective=False):
    """T timesteps, VS = padded vocab shard, VA = part of VS loaded early.
    reps>1 re-emits the whole body for marginal device-time measurement."""
    SC = _cdiv(T, 128)             # chunks of the attention (T) axis
    TC = SC                        # time chunks (projection M-tiles)
    s_last = T - (SC - 1) * 128
    assert s_last == 128, "schedule assumes T % 128 == 0"
    VC = _cdiv(VS, NV)
    GS = G + T                     # stacked [dec_Whh; A2] rows
    GSC = _cdiv(GS, 128)
    RZC = 2 * HC                   # fused r+z columns
    COL_S = SC                     # psA ssum column
    RZ0 = SC + 1                   # psA rz column base

    nc = bacc.Bacc("TRN2", target_bir_lowering=False, debug=False,
                   num_devices=n_cores)

    def din(name, shape, dt):
        return nc.dram_tensor(name, shape, dt, kind="ExternalInput").ap()

    whhe_t = din("whhe_t", [128, HC * G], E4)
    wihd_t = din("wihd_t", [128, HC * G], E4)
    wstk_t = din("wstk_t", [128, HC * GS], E4)
    wihe_t = din("wihe_t", [128, HC * G], BF)
    c1t = din("c1t", [128, HC * H], BF)
    c2t = din("c2t", [128, HC * H], BF)
    a1t = din("a1t", [128, HC * T], BF)
    embt = din("embt", [128, HC * T], BF)
    inpt = din("inpt", [128, HC * T], BF)
    iden_t = din("iden_t", [128, 128], F16)
    bias_e = din("bias_e", [128, GC], F32)
    bhn_e16_t = din("bhn_e16", [128, HC], F32)
    brzz_t = din("brzz", [128, 1 + RZC], F32)       # [0 | S*(bihd+bhhd)_rz]
    bhnxn_t = din("bhnxn16", [128, RZC], F32)       # [S*bhhd_n | S*bihd_n]
    bias_a = din("bias_a", [128, SC], F32)
    bias_c = din("bias_c", [128, HC], F32)
    out_wta = din("out_wta", [128, HC * VA], BF)
    out_wtb = din("out_wtb", [128, HC * (VS - VA)], BF)
    out_bb = din("out_bb", [1, VS], BF)

    out_d = nc.dram_tensor("out", [T, VS], F32, kind="ExternalOutput").ap()
    if debug_outs:
        dbg_enc = nc.dram_tensor("dbg_enc", [128, HC * T], BF,
                                 kind="ExternalOutput").ap()
        dbg_hdec = nc.dram_tensor("dbg_hdec", [128, HC * T], BF,
                                  kind="ExternalOutput").ap()

    with tile.TileContext(nc) as tc:
        # ----------------- persistent tiles -----------------
        cons_cm = tc.tile_pool(name="cons", bufs=1)
        cons = cons_cm.__enter__()
        enc_outsT = cons.tile([128, HC, T], BF, tag="enc_outsT")
        h_decT = cons.tile([128, HC, T], BF, tag="h_decT")
        m2_sb = cons.tile([128, TC, H], E4, tag="m2")
        # combined per-t PSUM preload tiles (single start=True mm per bank):
        # preA: [pa(SC) | 0 | brz(16)], preB: [pc(8) | bhn_d(8) | bxn_d(8)]
        preA16 = cons.tile([128, T, RZ0 + RZC], F16, tag="preA")
        preB16 = cons.tile([128, T, 3 * HC], F16, tag="preB")
        iden = cons.tile([128, 128], F16, tag="iden")
        be_sb = cons.tile([128, GC], F32, tag="be")
        bhn_e = cons.tile([128, HC], F32, tag="bhne")
        brzz = cons.tile([128, 1 + RZC], F32, tag="brzz")
        bhnxn = cons.tile([128, RZC], F32, tag="bhnxn")
        ba_sb = cons.tile([128, SC], F32, tag="ba")
        bc_sb = cons.tile([128, HC], F32, tag="bc")
        ones_bf = cons.tile([1, 128], BF, tag="ones_bf")
        onesSM = cons.tile([128, 128], F32, tag="onesSM")

        nc.sync.dma_start(iden[:], iden_t[:])
        nc.sync.dma_start(be_sb[:], bias_e[:])
        nc.sync.dma_start(bhn_e[:], bhn_e16_t[:])
        nc.sync.dma_start(brzz[:], brzz_t[:])
        nc.sync.dma_start(bhnxn[:], bhnxn_t[:])
        nc.sync.dma_start(ba_sb[:], bias_a[:])
        nc.sync.dma_start(bc_sb[:], bias_c[:])
        nc.vector.memset(ones_bf[:], 1.0)
        nc.vector.memset(onesSM[:], SM)

        # small per-step work tiles
        work_cm = tc.tile_pool(name="work", bufs=3)
        work = work_cm.__enter__()

        for rep in range(reps):
            # enc-phase tensors (freed after M2)
            encw_cm = tc.tile_pool(name="encw", bufs=1)
            encw = encw_cm.__enter__()
            whhe = encw.tile([128, HC, G], E4, tag="whhe")
            # [rz-x-part(16) | bhh_n bias bcast(8) | xn-x-part(8)], t-major;
            # cols 0:24 are the psum preload, cols 24:32 feed the t2 add
            encpre16 = encw.tile([128, T, GC + HC], F16, tag="encpre")
            c2 = encw.tile([128, HC, H], BF, tag="c2")
            nc.sync.dma_start(whhe[:], whhe_t[:])
            nc.sync.dma_start(c2[:], c2t[:])

            # ----------------- precompute phase -----------------
            with tc.tile_pool(name="pre", bufs=1) as pre, \
                 tc.tile_pool(name="prepsum", bufs=6, space="PSUM") as pps:
                wihe = pre.tile([128, HC, G], BF, tag="wihe")
                c1 = pre.tile([128, HC, H], BF, tag="c1")
                a1 = pre.tile([128, HC, T], BF, tag="a1")
                emb = pre.tile([128, HC, T], BF, tag="emb")
                inp = pre.tile([128, HC, T], BF, tag="inp")
                nc.sync.dma_start(wihe[:], wihe_t[:])
                nc.sync.dma_start(c1[:], c1t[:])
                nc.sync.dma_start(a1[:], a1t[:])
                nc.sync.dma_start(emb[:], embt[:])
                nc.sync.dma_start(inp[:], inpt[:])

                # encpre16 x-parts: cols 0:16 <- S*(xr,xz + biases),
                # cols 24:32 <- S*(xn + bih_n)  (wihe, bias_e S-scaled on host)
                for gc in range(GC):
                    dst = gc if gc < RZC else gc + HC
                    ps = pps.tile([128, T], F32, tag="pp")
                    for kc in range(HC):
                        nc.tensor.matmul(
                            ps[:], wihe[:, kc, gc * 128:(gc + 1) * 128],
                            inp[:, kc, :], start=(kc == 0), stop=(kc == HC - 1),
                            skip_group_check=True)
                    nc.vector.tensor_scalar(
                        out=encpre16[:, :, dst], in0=ps[:],
                        scalar1=be_sb[:, gc:gc + 1], scalar2=None, op0=ALU.add)
                # encpre16 cols 16:24 <- S*bhh_n broadcast along t
                for c in range(HC):
                    nc.vector.tensor_scalar(
                        out=encpre16[:, :, RZC + c], in0=inp[:, 0, :],
                        scalar1=0.0, scalar2=bhn_e[:, c:c + 1],
                        op0=ALU.mult, op1=ALU.add)

                # preB16[:, t, 0:HC] = (emb_seq @ C1.T).T + comb_b  (unscaled)
                for mc in range(HC):
                    ps = pps.tile([128, T], F32, tag="pp")
                    for kc in range(HC):
                        nc.tensor.matmul(
                            ps[:], c1[:, kc, mc * 128:(mc + 1) * 128],
                            emb[:, kc, :], start=(kc == 0), stop=(kc == HC - 1),
                            skip_group_check=True)
                    nc.vector.tensor_scalar(
                        out=preB16[:, :, mc], in0=ps[:],
                        scalar1=bc_sb[:, mc:mc + 1], scalar2=None, op0=ALU.add)
                # preB16 cols 8:24 <- [S*bhh_d_n | S*bih_d_n] broadcast
                for c in range(RZC):
                    nc.vector.tensor_scalar(
                        out=preB16[:, :, HC + c], in0=inp[:, 0, :],
                        scalar1=0.0, scalar2=bhnxn[:, c:c + 1],
                        op0=ALU.mult, op1=ALU.add)

                # preA16[:, t, 0:SC] = S*((emb_seq @ A1.T).T + attn_b)
                for sc in range(SC):
                    ps = pps.tile([128, T], F32, tag="pp")
                    for kc in range(HC):
                        nc.tensor.matmul(
                            ps[:], a1[:, kc, sc * 128:sc * 128 + 128],
                            emb[:, kc, :], start=(kc == 0), stop=(kc == HC - 1),
                            skip_group_check=True)
                    nc.vector.tensor_scalar(
                        out=preA16[:, :, sc], in0=ps[:],
                        scalar1=ba_sb[:, sc:sc + 1], scalar2=None,
                        op0=ALU.add)
                # preA16 cols SC:SC+17 <- [0 | S*(bih_d+bhh_d)_rz] broadcast
                for c in range(1 + RZC):
                    nc.vector.tensor_scalar(
                        out=preA16[:, :, SC + c], in0=inp[:, 0, :],
                        scalar1=0.0, scalar2=brzz[:, c:c + 1],
                        op0=ALU.mult, op1=ALU.add)

            # ----------------- encoder recurrence -----------------
            encp_cm = tc.tile_pool(name="encpsum", bufs=2, space="PSUM")
            encp = encp_cm.__enter__()

            # t = 0 (h = 0: W@h terms vanish; gxp is S-scaled with biases)
            rzt0 = work.tile([128, RZC], F32, tag="rzt")
            nc.scalar.activation(rzt0[:], encpre16[:, 0, 0:RZC], AF.Tanh,
                                 scale=0.5 / S)
            rz0 = work.tile([128, RZC], F32, tag="rz")
            nc.vector.tensor_scalar(out=rz0[:], in0=rzt0[:], scalar1=0.5,
                                    scalar2=0.5, op0=ALU.mult, op1=ALU.add)
            t10 = work.tile([128, HC], F32, tag="t1")
            nc.vector.tensor_tensor(out=t10[:], in0=rz0[:, 0:HC], in1=bhn_e[:],
                                    op=ALU.mult)
            t20 = work.tile([128, HC], F32, tag="t2")
            nc.vector.tensor_tensor(out=t20[:], in0=t10[:],
                                    in1=encpre16[:, 0, GC:GC + HC],
                                    op=ALU.add)
            n0 = work.tile([128, HC], F32, tag="n")
            nc.scalar.activation(n0[:], t20[:], AF.Tanh, scale=1.0 / S)
            d0 = work.tile([128, HC], F32, tag="d")
            nc.vector.tensor_scalar(out=d0[:], in0=n0[:], scalar1=-1.0,
                                    scalar2=None, op0=ALU.mult)
            zd0 = work.tile([128, HC], F32, tag="zd")
            nc.vector.tensor_tensor(out=zd0[:], in0=rz0[:, HC:RZC], in1=d0[:],
                                    op=ALU.mult)
            nc.vector.tensor_tensor(out=enc_outsT[:, 0:HC, 0], in0=n0[:],
                                    in1=zd0[:], op=ALU.add)

            for t in range(1, T):
                hsrc = enc_outsT[:, 0:HC, t - 1]
                ps = encp.tile([128, GC], F32, tag="pse")
                # single PSUM preload: rz cols <- S*(x-part+biases),
                # n cols <- S*bhh_n (one start=True per bank per step)
                nc.tensor.matmul(ps[:], iden[:], encpre16[:, t, 0:GC],
                                 start=True, stop=False, skip_group_check=True)
                # rz rows of Whh @ h, kc-outer in halves
                for khalf in range(2):
                    for kc in range(khalf * HH, khalf * HH + HH):
                        for c in range(RZC):
                            nc.tensor.matmul(
                                ps[:, c:c + 1],
                                whhe[:, kc, c * 128:(c + 1) * 128],
                                hsrc[:, kc:kc + 1],
                                start=False, stop=(kc == HC - 1),
                                skip_group_check=True)
                # rz chain (overlaps the n-block stream)
                rzt = work.tile([128, RZC], F32, tag="rzt")
                nc.scalar.activation(rzt[:], ps[:, 0:RZC], AF.Tanh,
                                     scale=0.5 / S)
                rz = work.tile([128, RZC], F32, tag="rz")
                nc.vector.tensor_scalar(out=rz[:], in0=rzt[:], scalar1=0.5,
                                        scalar2=0.5, op0=ALU.mult, op1=ALU.add)
                zh = work.tile([128, HC], F32, tag="zh")
                nc.vector.tensor_tensor(out=zh[:], in0=rz[:, HC:RZC],
                                        in1=hsrc, op=ALU.mult)
                omz = work.tile([128, HC], F32, tag="omz")
                nc.vector.tensor_scalar(out=omz[:], in0=rz[:, HC:RZC],
                                        scalar1=-1.0, scalar2=1.0,
                                        op0=ALU.mult, op1=ALU.add)
                # n rows of Whh @ h in two output halves; gate tail staged so
                # h[0:HH] lands before the stream ends
                nhalf_t = []
                for nh in range(2):
                    for c in range(RZC + nh * HH, RZC + (nh + 1) * HH):
                        for kc in range(HC):
                            nc.tensor.matmul(
                                ps[:, c:c + 1],
                                whhe[:, kc, c * 128:(c + 1) * 128],
                                hsrc[:, kc:kc + 1],
                                start=False, stop=(kc == HC - 1),
                                skip_group_check=True)
                    sl = slice(nh * HH, (nh + 1) * HH)
                    nsl = slice(RZC + nh * HH, RZC + (nh + 1) * HH)
                    t1 = work.tile([128, HH], F32, tag=f"t1{nh}")
                    nc.vector.tensor_tensor(out=t1[:], in0=rz[:, sl],
                                            in1=ps[:, nsl], op=ALU.mult)
                    t2 = work.tile([128, HH], F32, tag=f"t2{nh}")
                    nc.vector.tensor_tensor(
                        out=t2[:], in0=t1[:],
                        in1=encpre16[:, t, GC + nh * HH:GC + (nh + 1) * HH],
                        op=ALU.add)
                    n_ = work.tile([128, HH], F32, tag=f"n{nh}")
                    nc.scalar.activation(n_[:], t2[:], AF.Tanh, scale=1.0 / S)
                    nhalf_t.append((sl, n_))
                    # first half's p/h are emitted right after its tanh so the
                    # DVE FIFO runs them before the second half's t1/t2
                    p_ = work.tile([128, HH], F32, tag=f"p{nh}")
                    nc.vector.tensor_tensor(out=p_[:], in0=n_[:],
                                            in1=omz[:, sl], op=ALU.mult)
                    nc.vector.tensor_tensor(
                        out=enc_outsT[:, nh * HH:(nh + 1) * HH, t],
                        in0=p_[:], in1=zh[:, sl], op=ALU.add)

            encp_cm.__exit__(None, None, None)

            # ----------------- M2 = enc_outs @ C2.T  (SM-scaled fp8) -------
            m2p_cm = tc.tile_pool(name="m2psum", bufs=4, space="PSUM")
            m2p = m2p_cm.__enter__()
            for tc_i in range(TC):
                for n0_ in range(0, H, NV):
                    ps = m2p.tile([128, NV], F32, tag="m2p")
                    for kc in range(HC):
                        nc.tensor.matmul(
                            ps[:],
                            enc_outsT[:, kc, tc_i * 128:(tc_i + 1) * 128],
                            c2[:, kc, n0_:n0_ + NV],
                            start=(kc == 0), stop=(kc == HC - 1),
                            skip_group_check=True)
                    nc.vector.tensor_copy(m2_sb[:, tc_i, n0_:n0_ + NV],
                                          ps[:])
            m2p_cm.__exit__(None, None, None)
            encw_cm.__exit__(None, None, None)

            # dec-phase stationary weights + both projection halves (their
            # DMAs hide under the decoder)
            oww_cm = tc.tile_pool(name="oww", bufs=1)
            oww = oww_cm.__enter__()
            owa = oww.tile([128, HC, VA], BF, tag="owa")
            nc.sync.dma_start(owa[:], out_wta[:])

            owbp_cm = tc.tile_pool(name="owbp", bufs=1)
            owbp = owbp_cm.__enter__()
            owb = owbp.tile([128, HC, VS - VA], BF, tag="owb")
            outb_sb = owbp.tile([1, VS], BF, tag="outb")

            decw_cm = tc.tile_pool(name="decw", bufs=1)
            decw = decw_cm.__enter__()
            wihd = decw.tile([128, HC, G], E4, tag="wihd")
            wstk = decw.tile([128, HC, GS], E4, tag="wstk")
            nc.sync.dma_start(wihd[:], wihd_t[:])
            nc.sync.dma_start(wstk[:], wstk_t[:])
            nc.sync.dma_start(owb[:], out_wtb[:])
            nc.sync.dma_start(outb_sb[:], out_bb[:])

            # ----------------- decoder recurrence -----------------
            # psA: [att(SC) | ssum(1) | rz(16)]; psB: [q(8) | nh(8) | nx(8)]
            # All columns preloaded via identity matmuls (start=True), then
            # every accumulation is start=False onto the preloaded content.
            decpA_cm = tc.tile_pool(name="decpsA", bufs=2, space="PSUM")
            decpA = decpA_cm.__enter__()
            decpB_cm = tc.tile_pool(name="decpsB", bufs=2, space="PSUM")
            decpB = decpB_cm.__enter__()

            for t in range(T):
                psA = decpA.tile([128, RZ0 + RZC], F32, tag="psA")
                psB = decpB.tile([128, 3 * HC], F32, tag="psB")

                if t == 0:
                    hsrc = enc_outsT[:, 0:HC, T - 1]
                else:
                    hsrc = h_decT[:, 0:HC, t - 1]

                # PSUM preloads: one start=True mm per bank (consecutive ->
                # shared iden LDW). psA <- [pa | 0 | brz]; psB <- [pc | bhnxn]
                nc.tensor.matmul(psA[:], iden[:], preA16[:, t, :],
                                 start=True, stop=False, skip_group_check=True)
                nc.tensor.matmul(psB[:], iden[:], preB16[:, t, :],
                                 start=True, stop=False, skip_group_check=True)

                # Whh rz, kc 0:4 (only needs first half of h)
                for kc in range(HH):
                    for c in range(RZC):
                        nc.tensor.matmul(
                            psA[:, RZ0 + c:RZ0 + c + 1],
                            wstk[:, kc, c * 128:(c + 1) * 128],
                            hsrc[:, kc:kc + 1],
                            start=False, stop=False, skip_group_check=True)
                # A2 attention rows @ h (kc-outer; stop on last kc)
                for kc in range(HC):
                    for st in range(GC, GSC):
                        lo = st * 128
                        sc = st - GC
                        nc.tensor.matmul(
                            psA[:, sc:sc + 1],
                            wstk[:, kc, lo:lo + 128],
                            hsrc[:, kc:kc + 1],
                            start=False, stop=(kc == HC - 1),
                            skip_group_check=True)
                # softmax: exp with accumulate (psA attn has pa preloaded)
                e_bf = work.tile([128, SC], BF, tag="e")
                acc = work.tile([128, 1], F32, tag="acc")
                nc.scalar.activation(e_bf[:], psA[:, 0:SC], AF.Exp,
                                     scale=1.0 / S, accum_out=acc[:])
                # Whh rz kc 4,5
                for kc in range(HH, HH + 2):
                    for c in range(RZC):
                        nc.tensor.matmul(
                            psA[:, RZ0 + c:RZ0 + c + 1],
                            wstk[:, kc, c * 128:(c + 1) * 128],
                            hsrc[:, kc:kc + 1],
                            start=False, stop=False, skip_group_check=True)
                # SM-scaled cross-partition exp-sum onto preloaded 0
                nc.tensor.matmul(psA[:, COL_S:COL_S + 1], onesSM[:], acc[:],
                                 start=False, stop=True, skip_group_check=True)
                rs_col = work.tile([128, 1], F32, tag="rscol")
                nc.vector.reciprocal(rs_col[:], psA[:, COL_S:COL_S + 1])
                e_nrm = work.tile([128, SC], BF, tag="enrm")
                nc.vector.tensor_scalar(out=e_nrm[:], in0=e_bf[:],
                                        scalar1=rs_col[:], scalar2=None,
                                        op0=ALU.mult)
                # Whh rz kc 6,7
                for kc in range(HH + 2, HC):
                    for c in range(RZC):
                        nc.tensor.matmul(
                            psA[:, RZ0 + c:RZ0 + c + 1],
                            wstk[:, kc, c * 128:(c + 1) * 128],
                            hsrc[:, kc:kc + 1],
                            start=False, stop=False, skip_group_check=True)

                comb_bf = work.tile([128, HC], BF, tag="comb")
                # q = e_nrm @ M2 (first half) onto preloaded pc -> relu
                for mh in range(2):
                    for mc in range(mh * HH, (mh + 1) * HH):
                        for tc_i in range(TC):
                            nc.tensor.matmul(
                                psB[:, mc:mc + 1],
                                m2_sb[:, tc_i, mc * 128:(mc + 1) * 128],
                                e_nrm[:, tc_i:tc_i + 1],
                                start=False, stop=(tc_i == TC - 1),
                                skip_group_check=True)
                    nc.scalar.activation(
                        comb_bf[:, mh * HH:(mh + 1) * HH],
                        psB[:, mh * HH:(mh + 1) * HH], AF.Relu)
                    if mh == 0:
                        # Whh n rows (psB nh cols, bias preloaded)
                        for c in range(HC):
                            gc = RZC + c
                            for kc in range(HC):
                                nc.tensor.matmul(
                                    psB[:, HC + c:HC + c + 1],
                                    wstk[:, kc, gc * 128:(gc + 1) * 128],
                                    hsrc[:, kc:kc + 1],
                                    start=False, stop=(kc == HC - 1),
                                    skip_group_check=True)

                # Wih @ comb: rz cols kc-outer (comb halves arrive in order)
                for kc in range(HC):
                    for c in range(RZC):
                        nc.tensor.matmul(
                            psA[:, RZ0 + c:RZ0 + c + 1],
                            wihd[:, kc, c * 128:(c + 1) * 128],
                            comb_bf[:, kc:kc + 1],
                            start=False, stop=(kc == HC - 1),
                            skip_group_check=True)
                # rz chain (overlaps the Wih n stream)
                rzt = work.tile([128, RZC], F32, tag="rzt")
                nc.scalar.activation(rzt[:], psA[:, RZ0:RZ0 + RZC], AF.Tanh,
                                     scale=0.5 / S)
                rz = work.tile([128, RZC], F32, tag="rz")
                nc.vector.tensor_scalar(out=rz[:], in0=rzt[:], scalar1=0.5,
                                        scalar2=0.5, op0=ALU.mult, op1=ALU.add)
                zh = work.tile([128, HC], F32, tag="zh")
                nc.vector.tensor_tensor(out=zh[:], in0=rz[:, HC:RZC],
                                        in1=hsrc, op=ALU.mult)
                omz = work.tile([128, HC], F32, tag="omz")
                nc.vector.tensor_scalar(out=omz[:], in0=rz[:, HC:RZC],
                                        scalar1=-1.0, scalar2=1.0,
                                        op0=ALU.mult, op1=ALU.add)
                t1 = work.tile([128, HC], F32, tag="t1")
                nc.vector.tensor_tensor(out=t1[:], in0=rz[:, 0:HC],
                                        in1=psB[:, HC:2 * HC], op=ALU.mult)
                # Wih n rows in two output halves + staged gate tail
                tails = []
                for nh in range(2):
                    for c in range(nh * HH, (nh + 1) * HH):
                        gc = RZC + c
                        for kc in range(HC):
                            nc.tensor.matmul(
                                psB[:, 2 * HC + c:2 * HC + c + 1],
                                wihd[:, kc, gc * 128:(gc + 1) * 128],
                                comb_bf[:, kc:kc + 1],
                                start=False, stop=(kc == HC - 1),
                                skip_group_check=True)
                    sl = slice(nh * HH, (nh + 1) * HH)
                    t2 = work.tile([128, HH], F32, tag=f"t2{nh}")
                    nc.vector.tensor_tensor(
                        out=t2[:], in0=t1[:, sl],
                        in1=psB[:, 2 * HC + nh * HH:2 * HC + (nh + 1) * HH],
                        op=ALU.add)
                    n_ = work.tile([128, HH], F32, tag=f"n{nh}")
                    nc.scalar.activation(n_[:], t2[:], AF.Tanh, scale=1.0 / S)
                    tails.append((nh, sl, n_))
                for nh, sl, n_ in tails:
                    p_ = work.tile([128, HH], F32, tag=f"p{nh}")
                    nc.vector.tensor_tensor(out=p_[:], in0=n_[:],
                                            in1=omz[:, sl], op=ALU.mult)
                    nc.vector.tensor_tensor(
                        out=h_decT[:, nh * HH:(nh + 1) * HH, t],
                        in0=p_[:], in1=zh[:, sl], op=ALU.add)

            decpB_cm.__exit__(None, None, None)
            decpA_cm.__exit__(None, None, None)
            decw_cm.__exit__(None, None, None)

            # ----------------- projection + log_softmax -----------------
            projw_cm = tc.tile_pool(name="projw", bufs=1)
            projw = projw_cm.__enter__()
            logit16 = projw.tile([128, TC, VS], F16, tag="logit16")
            sacc = projw.tile([128, TC, VC], F32, tag="sacc")
            s_loc = projw.tile([128, TC], F32, tag="sloc")
            logz = projw.tile([128, TC], F32, tag="logz")
            nc.vector.memset(sacc[:], 0.0)

            def w_slice(j):
                v0 = j * NV
                nv = min(NV, VS - v0)
                if v0 < VA:
                    return owa, v0, nv
                return owb, v0 - VA, nv

            pj1_cm = tc.tile_pool(name="pj1", bufs=4, space="PSUM")
            pj1 = pj1_cm.__enter__()
            scr_cm = tc.tile_pool(name="scr", bufs=3)
            scr = scr_cm.__enter__()

            # single matmul pass: psum -> fp16 logits in SBUF -> exp-sum
            for m in range(TC):
                for j in range(VC):
                    src, off, nv = w_slice(j)
                    v0 = j * NV
                    ps = pj1.tile([128, NV], F32, tag="pj1")
                    nc.tensor.matmul(ps[:, 0:nv], ones_bf[0:1, :],
                                     outb_sb[0:1, v0:v0 + nv],
                                     start=True, stop=False,
                                     skip_group_check=True)
                    for kc in range(HC):
                        nc.tensor.matmul(
                            ps[:, 0:nv],
                            h_decT[:, kc, m * 128:(m + 1) * 128],
                            src[:, kc, off:off + nv],
                            start=False, stop=(kc == HC - 1),
                            skip_group_check=True)
                    lg = logit16[:, m, v0:v0 + nv]
                    nc.vector.tensor_copy(lg, ps[:, 0:nv])
                    escr = scr.tile([128, NV], BF, tag="escr")
                    nc.scalar.activation(
                        escr[:, 0:nv],
                        lg, AF.Exp, accum_out=sacc[:, m, j:j + 1])
            nc.vector.reduce_sum(s_loc[:], sacc[:], axis=mybir.AxisListType.X)

            with tc.tile_pool(name="dram", bufs=1, space="DRAM") as dram:
                s_tot = projw.tile([128, TC], F32, tag="stot")
                if no_collective:
                    nc.vector.tensor_copy(s_tot[:], s_loc[:])
                else:
                    ib = dram.tile([128, TC], F32)
                    ob = dram.tile([128, TC], F32)
                    nc.gpsimd.dma_start(ib[:], s_loc[:])
                    nc.gpsimd.collective_compute(
                        "AllReduce", ALU.add,
                        replica_groups=[list(range(n_cores))],
                        ins=[ib.opt()], outs=[ob.opt()])
                    nc.sync.dma_start(s_tot[:], ob[:])
                nc.scalar.activation(logz[:], s_tot[:], AF.Ln)

                for m in range(TC):
                    for j in range(VC):
                        v0 = j * NV
                        nv = min(NV, VS - v0)
                        ot = scr.tile([128, NV], F32, tag="oscr")
                        nc.vector.tensor_scalar(
                            out=ot[:, 0:nv],
                            in0=logit16[:, m, v0:v0 + nv],
                            scalar1=logz[:, m:m + 1], scalar2=None,
                            op0=ALU.subtract)
                        nc.sync.dma_start(
                            out_d[m * 128:(m + 1) * 128, v0:v0 + nv],
                            ot[:, 0:nv])

            if debug_outs:
                nc.sync.dma_start(dbg_enc[:],
                                  enc_outsT[:].rearrange("p a b -> p (a b)"))
                nc.sync.dma_start(dbg_hdec[:],
                                  h_decT[:].rearrange("p a b -> p (a b)"))

            scr_cm.__exit__(None, None, None)
            pj1_cm.__exit__(None, None, None)
            projw_cm.__exit__(None, None, None)
            owbp_cm.__exit__(None, None, None)
            oww_cm.__exit__(None, None, None)

        work_cm.__exit__(None, None, None)
        cons_cm.__exit__(None, None, None)

    nc.compile()
    return nc


# ---------------------------------------------------------------------------
# host side
# ---------------------------------------------------------------------------

def _tiles(M, dt=BF16):
    """M [rows, H] -> lhsT tile layout [128, HC*rows] (M.T tiled)."""
    rows = M.shape[0]
    return np.ascontiguousarray(
        M.T.reshape(HC, 128, rows).transpose(1, 0, 2).reshape(128, HC * rows)
    ).astype(dt)


def _cols(v, dt=np.float32):
    """v [C*128] -> [128, C] column layout."""
    C = v.shape[0] // 128
    return np.ascontiguousarray(v.reshape(C, 128).T).astype(dt)


_PROG_CACHE = {}


def _get_program(T, VS, VA, reps=1):
    key = (T, VS, VA, reps)
    if key not in _PROG_CACHE:
        _PROG_CACHE[key] = build_program(T, VS, VA, reps=reps)
    return _PROG_CACHE[key]


def prepare_inputs(inputs, T, VS, VA):
    f32 = np.float32
    inp = np.asarray(inputs["input_seq"], f32)[:, 0, :]      # [T, H]
    target = np.asarray(inputs["target"]).astype(np.int64)[:, 0]
    emb_dec = np.asarray(inputs["emb_dec"], f32)
    toks = np.concatenate([[SOS], target[:-1]])
    emb_seq = emb_dec[toks]                                   # [T, H]

    attn_W = np.asarray(inputs["attn_W"], f32)
    A1, A2 = attn_W[:, :H], attn_W[:, H:]
    comb_W = np.asarray(inputs["comb_W"], f32)
    C1, C2 = comb_W[:, :H], comb_W[:, H:]
    enc_bih = np.asarray(inputs["enc_bih"], f32)
    enc_bhh = np.asarray(inputs["enc_bhh"], f32)
    dec_bih = np.asarray(inputs["dec_bih"], f32)
    dec_bhh = np.asarray(inputs["dec_bhh"], f32)
    attn_b = np.asarray(inputs["attn_b"], f32)
    comb_b = np.asarray(inputs["comb_b"], f32)
    out_W = np.asarray(inputs["out_W"], f32)
    out_b = np.asarray(inputs["out_b"], f32)

    SC = _cdiv(T, 128)
    stk = np.concatenate([np.asarray(inputs["dec_Whh"], f32), A2], axis=0)

    ve = np.concatenate([(enc_bih + enc_bhh)[:2 * H], enc_bih[2 * H:]])
    attn_b_pad = np.zeros(SC * 128, f32)
    attn_b_pad[:T] = attn_b

    brzz = np.concatenate(
        [np.zeros((128, 1), f32), _cols(S * (dec_bih + dec_bhh)[:2 * H])],
        axis=1)

    shared = {
        # fp8 S-scaled recurrence weights
        "whhe_t": _tiles(S * np.asarray(inputs["enc_Whh"], f32), FP8),
        "wihd_t": _tiles(S * np.asarray(inputs["dec_Wih"], f32), FP8),
        "wstk_t": _tiles(S * stk, FP8),
        # bf16 batched-phase weights (S folded in where the consumer is
        # S-scaled; exact: S, SM are powers of two)
        "wihe_t": _tiles(S * np.asarray(inputs["enc_Wih"], f32)),
        "c1t": _tiles(C1),
        "c2t": _tiles(SM * C2),
        "a1t": _tiles(S * A1),
        "embt": _tiles(emb_seq),
        "inpt": _tiles(inp),
        "iden_t": np.eye(128, dtype=F16NP),
        "bias_e": _cols(S * ve),
        "bhn_e16": _cols(S * enc_bhh[2 * H:]),
        "brzz": brzz,
        "bhnxn16": np.concatenate(
            [_cols(S * dec_bhh[2 * H:]), _cols(S * dec_bih[2 * H:])],
            axis=1),
        "bias_a": _cols(S * attn_b_pad),
        "bias_c": _cols(comb_b),
    }

    V = out_W.shape[0]
    in_maps = []
    for c in range(N_CORES):
        vlo = c * VS
        vhi = min(V, vlo + VS)
        Wsh = np.zeros((VS, H), f32)
        bsh = np.full(VS, -1e30, f32)
        if vhi > vlo:
            Wsh[:vhi - vlo] = out_W[vlo:vhi]
            bsh[:vhi - vlo] = out_b[vlo:vhi]
        wt = _tiles(Wsh)                    # [128, HC*VS]
        wt3 = wt.reshape(128, HC, VS)
        m = dict(shared)
        m["out_wta"] = np.ascontiguousarray(wt3[:, :, :VA]).reshape(128, HC * VA)
        m["out_wtb"] = np.ascontiguousarray(wt3[:, :, VA:]).reshape(
            128, HC * (VS - VA))
        m["out_bb"] = bsh.astype(BF16)[None, :]
        in_maps.append(m)
    return in_maps


def run(inputs, T=256, VS=None, VA=None, trace=False):
    V = np.asarray(inputs["out_W"]).shape[0]
    if VS is None:
        VS = _cdiv(_cdiv(V, N_CORES), NV) * NV   # 6656 for V=50257
    if VA is None:
        VA = max(NV, (VS // (2 * NV)) * NV)
    nc = _get_program(T, VS, VA)
    in_maps = prepare_inputs(inputs, T, VS, VA)
    res = run_bass_kernel_spmd(nc, in_maps, core_ids=list(range(N_CORES)),
                               trace=trace)
    parts = []
    for c in range(N_CORES):
        vlo = c * VS
        vhi = min(V, vlo + VS)
        if vhi <= vlo:
            continue
        parts.append(res.results[c]["out"][:, :vhi - vlo])
    full = np.concatenate(parts, axis=1).astype(np.float32)
    return full.reshape(T, 1, V), res


def kernel(**inputs):
    out, _ = run(inputs, T=256)
    return out
